# revision 9
# baseline (speedup 1.0000x reference)
# Trainium2 Bass kernel for FJSP actor head (gnn_message_passing).
#
# Math (per batch b):
#   job_emb = ops_emb[b, next_op[b], :]                  [50, 128]  (gather)
#   u_j = job_emb @ W1[:128]   v_m = ma_emb[b] @ W1[128:]
#   h1[j,m] = relu(u_j + v_m + b1)            -> 2000 pairs (+1 noop)
#   h2 = relu(h1 @ W2 + b2);  logit = h2 @ W3 + b3
#
# Device strategy (pure data parallel over batch, 32 batches/core):
#   * ops/ma passed to the device pre-cast to bf16; all transposes are
#     done by the DMA xbar (dma_start_transpose) -- no PE transposes.
#   * Pairwise broadcast u_j + v_m is ONE matmul per 512-col chunk:
#     lhsT = jvp (rows 0-49 u, 64-103 v) [104, 128], rhs = constant 0/1
#     selection matrix smat.  b1/b2 are folded into the relu evacuation
#     biases; the noop logit (col 0) is computed on host.
#   * 3-stage software pipeline per batch on the PE queue:
#       S x4 (b) | W2 x4 (b-1) | W3 x4 (b-2, col-tiled -> concurrent).
#     The scalar+vector engines (PSUM evacuation, the true bottleneck)
#     carry a fixed, balanced split of relu1/relu2/logit copies.
#   * PSUM: h1 2x[128,1024] (ci0/1 S-out) + midA 1x[128,1024] (ci0/1
#     W2-out) + stream 2x[128,512] (ci2/3: the W2 output reuses the
#     bank its own relu1 just vacated, so no cross-batch chain) +
#     shared 1x[128,512] (proj + logits) = 8 banks.
#   * DMA count kept low (Tile recycles completion-semaphore slots and
#     chains unrelated DMAs): weights packed into one wall tensor,
#     biases into one, gathers trickled just-in-time.

import numpy as np
from contextlib import ExitStack

import concourse.bass as bass
import concourse.mybir as mybir
import concourse.tile as tile
from concourse import bacc
from concourse.bass_utils import run_bass_kernel_spmd

BS, N_OPS, N_JOBS, N_MA, E, H = 256, 2000, 50, 40, 128, 128
NCORES = 8
BPC = BS // NCORES            # 32 batches per core
NPAIR = N_JOBS * N_MA + 1     # 2001 logits per batch (col 0 = noop)
NPAD = 2048                   # padded logit row (cols 2001:2048 are junk)
PB = 64                       # gather rows reserved per batch (50 real + 14 pad)
NCHUNK = BPC * PB // 128      # 16 gather chunks of 128 rows
R_V0 = 64                     # v_m rows 64..103  (u_j rows at 0..49)
KJV = 104                     # S matrix rows
N3 = 472                      # trimmed width of the last 512-col chunk
NB = 512 + N3                 # h1b/Ab width

f32 = mybir.dt.float32
bf16 = mybir.dt.bfloat16

Relu = mybir.ActivationFunctionType.Relu
Add = mybir.AluOpType.add
Max = mybir.AluOpType.max


def _build_smat() -> np.ndarray:
    S = np.zeros((KJV, NPAD), np.float32)
    for j in range(N_JOBS):
        S[j, 1 + j * N_MA: 1 + (j + 1) * N_MA] = 1.0
    for m in range(N_MA):
        S[R_V0 + m, 1 + m: NPAIR: N_MA] = 1.0
    return S


def _build_module() -> bass.Bass:
    nc = bacc.Bacc("TRN2", target_bir_lowering=False, debug=False)
    ops = nc.dram_tensor("ops", [BPC * N_OPS, E], bf16, kind="ExternalInput")
    ma = nc.dram_tensor("ma", [BPC * N_MA, E], bf16, kind="ExternalInput")
    idx = nc.dram_tensor("idx", [128, NCHUNK], mybir.dt.int32, kind="ExternalInput")
    smat = nc.dram_tensor("smat", [KJV, NPAD], bf16, kind="ExternalInput")
    # weight wall: cols 0:128 Wj, 128:256 Wm, 256:384 W2, 384 w3
    wall = nc.dram_tensor("wall", [128, 385], bf16, kind="ExternalInput")
    bvec = nc.dram_tensor("bvec", [128, 2], f32, kind="ExternalInput")
    out = nc.dram_tensor("out", [BPC, NPAD], f32, kind="ExternalOutput")

    with tile.TileContext(nc) as tc, ExitStack() as ctx:
        singles = ctx.enter_context(tc.tile_pool(name="singles", bufs=1))

        # ---- preamble: idx + maT xbar on sync (jT xbars follow there);
        # wall/bias/smat on the scalar HWDGE queue (idle until relu time).
        idx_s = singles.tile([128, NCHUNK], mybir.dt.int32)
        nc.sync.dma_start(out=idx_s[:], in_=idx[:])
        maT = singles.tile([128, BPC * N_MA], bf16)
        nc.sync.dma_start_transpose(out=maT[:], in_=ma[:])

        wall_s = singles.tile([128, 385], bf16)
        nc.scalar.dma_start(out=wall_s[:], in_=wall[:])
        smat_s = singles.tile([KJV, NPAD], bf16)
        nc.scalar.dma_start(out=smat_s[:], in_=smat[:])
        bb_s = singles.tile([128, 2], f32)
        nc.scalar.dma_start(out=bb_s[:], in_=bvec[:])

        wj_s = wall_s[:, 0:128]
        wm_s = wall_s[:, 128:256]
        w2_s = wall_s[:, 256:384]
        w3_s = wall_s[:, 384:385]
        b1_s = bb_s[:, 0:1]
        b2_s = bb_s[:, 1:2]

        # ---- gathers trickled just-in-time on gpsimd ----
        grows_pool = ctx.enter_context(tc.tile_pool(name="growsp", bufs=6))
        grows: dict = {}

        def emit_gather(c):
            g = grows_pool.tile([128, E], bf16, tag="grows", name=f"grows{c}")
            nc.gpsimd.indirect_dma_start(
                out=g[:], out_offset=None, in_=ops[:],
                in_offset=bass.IndirectOffsetOnAxis(ap=idx_s[:, c:c + 1], axis=0),
            )
            grows[c] = g

        for c in range(4):
            emit_gather(c)

        # ---- pools ----
        jt_pool = ctx.enter_context(tc.tile_pool(name="jt", bufs=3))
        jv_pool = ctx.enter_context(tc.tile_pool(name="jvp", bufs=3))
        h1_ps = ctx.enter_context(tc.tile_pool(name="h1ps", bufs=1, space="PSUM"))
        midA_ps = ctx.enter_context(tc.tile_pool(name="midAps", bufs=1, space="PSUM"))
        str_ps = ctx.enter_context(tc.tile_pool(name="strps", bufs=3, space="PSUM"))
        sh_ps = ctx.enter_context(tc.tile_pool(name="shps", bufs=1, space="PSUM"))
        a_pool = ctx.enter_context(tc.tile_pool(name="ap", bufs=4))
        h2_pool = ctx.enter_context(tc.tile_pool(name="h2s", bufs=8))
        st_pool = ctx.enter_context(tc.tile_pool(name="st", bufs=6))

        # PE warm-up during the initial DMA window (HAM un-throttle)
        warm = singles.tile([128, 512], bf16)
        nc.vector.memset(warm[:].bitcast(mybir.dt.uint16), 0)
        for _ in range(6):
            wp = h1_ps.tile([128, 1024], f32, tag="h1", name="warmps")
            nc.tensor.matmul(out=wp[:, 0:512], lhsT=warm[:, 0:128], rhs=warm[:],
                             start=True, stop=True)

        # per-chunk setup: xbar-transpose the gathered rows, then project
        state = {"jvp": {}, "A": {}, "H2": {}}

        def emit_chunk_setup(c):
            jT = jt_pool.tile([128, 128], bf16, tag="jt", name=f"jt{c}")
            nc.sync.dma_start_transpose(out=jT[:], in_=grows[c][:])
            pj = sh_ps.tile([128, 512], f32, tag="sh", name=f"pj{c}")
            jvp = jv_pool.tile([128, 256], bf16, tag="jv", name=f"jv{c}")
            for sub in range(2):
                nc.tensor.matmul(out=pj[0:PB, 128 * sub:128 * (sub + 1)],
                                 lhsT=jT[:, sub * PB:(sub + 1) * PB],
                                 rhs=wj_s, start=True, stop=True)
            for sub in range(2):
                mcol = (2 * c + sub) * N_MA
                nc.tensor.matmul(out=pj[R_V0:R_V0 + N_MA, 128 * sub:128 * (sub + 1)],
                                 lhsT=maT[:, mcol:mcol + N_MA],
                                 rhs=wm_s, start=True, stop=True)
            # one evacuation for the whole projection block (DVE)
            nc.vector.tensor_copy(out=jvp[0:KJV, :], in_=pj[0:KJV, 0:256])
            state["jvp"][c] = jvp
            del grows[c]

        emit_chunk_setup(0)

        def emit_S(b):
            """S-matmuls ci0/1 -> h1a [1024]; ci2 -> strX, ci3 -> strY.
            relu1: h1a on ACT (1024), strX+strY on DVE (512+472)."""
            c, sub = b // 2, b % 2
            jvp = state["jvp"][b // 2]
            lhs = jvp[0:KJV, 128 * sub:128 * (sub + 1)]
            h1a = h1_ps.tile([128, 1024], f32, tag="h1", name=f"h1a{b}")
            strX = str_ps.tile([128, 512], f32, tag="str", name=f"sX{b}")
            strY = str_ps.tile([128, 512], f32, tag="str", name=f"sY{b}")
            Aa = a_pool.tile([128, 1024], bf16, tag="A", name=f"Aa{b}")
            Ab = a_pool.tile([128, 1024], bf16, tag="A", name=f"Ab{b}")
            nc.tensor.matmul(out=h1a[:, 0:512], lhsT=lhs,
                             rhs=smat_s[:, 0:512], start=True, stop=True)
            nc.tensor.matmul(out=h1a[:, 512:1024], lhsT=lhs,
                             rhs=smat_s[:, 512:1024], start=True, stop=True)
            nc.scalar.activation(out=Aa[:], in_=h1a[:], func=Relu, bias=b1_s)
            nc.tensor.matmul(out=strX[:], lhsT=lhs,
                             rhs=smat_s[:, 1024:1536], start=True, stop=True)
            nc.tensor.matmul(out=strY[:, 0:N3], lhsT=lhs,
                             rhs=smat_s[:, 1536:1536 + N3], start=True, stop=True)
            nc.vector.tensor_scalar(out=Ab[:, 0:512], in0=strX[:],
                                    scalar1=b1_s, scalar2=0.0, op0=Add, op1=Max)
            nc.vector.tensor_scalar(out=Ab[:, 512:NB], in0=strY[:, 0:N3],
                                    scalar1=b1_s, scalar2=0.0, op0=Add, op1=Max)
            state["A"][b] = (Aa, Ab, strX, strY)

        def emit_W2(b):
            """W2: ci2/ci3 back into the stream banks their relu1 vacated;
            ci0/1 into midA.  relu2: midA on ACT (1024), streams on DVE."""
            Aa, Ab, strX, strY = state["A"][b]
            midA = midA_ps.tile([128, 1024], f32, tag="midA", name=f"mA{b}")
            H2a = h2_pool.tile([128, 1024], bf16, tag="H2", name=f"H2a{b}")
            H2b = h2_pool.tile([128, 512], bf16, tag="H2", name=f"H2b{b}")
            H2c = h2_pool.tile([128, 512], bf16, tag="H2", name=f"H2c{b}")
            nc.tensor.matmul(out=strX[:], lhsT=w2_s,
                             rhs=Ab[:, 0:512], start=True, stop=True)
            nc.tensor.matmul(out=strY[:, 0:N3], lhsT=w2_s,
                             rhs=Ab[:, 512:NB], start=True, stop=True)
            nc.tensor.matmul(out=midA[:, 0:512], lhsT=w2_s,
                             rhs=Aa[:, 0:512], start=True, stop=True)
            nc.tensor.matmul(out=midA[:, 512:1024], lhsT=w2_s,
                             rhs=Aa[:, 512:1024], start=True, stop=True)
            nc.vector.tensor_scalar(out=H2b[:], in0=strX[:],
                                    scalar1=b2_s, scalar2=0.0, op0=Add, op1=Max)
            nc.vector.tensor_scalar(out=H2c[:, 0:N3], in0=strY[:, 0:N3],
                                    scalar1=b2_s, scalar2=0.0, op0=Add, op1=Max)
            nc.scalar.activation(out=H2a[:], in_=midA[:], func=Relu, bias=b2_s)
            state["H2"][b] = (H2a, H2b, H2c)
            del state["A"][b]

        def emit_W3(b):
            H2a, H2b, H2c = state["H2"][b]
            lg = sh_ps.tile([128, 512], f32, tag="sh", name=f"lg{b}")
            nc.tensor.matmul(out=lg[0:1, 0:512], lhsT=w3_s,
                             rhs=H2a[:, 0:512], start=True, stop=True,
                             tile_position=(0, 0))
            nc.tensor.matmul(out=lg[32:33, 0:512], lhsT=w3_s,
                             rhs=H2a[:, 512:1024], start=True, stop=True,
                             tile_position=(0, 32))
            nc.tensor.matmul(out=lg[64:65, 0:512], lhsT=w3_s,
                             rhs=H2b[:], start=True, stop=True,
                             tile_position=(0, 64))
            nc.tensor.matmul(out=lg[96:97, 0:N3], lhsT=w3_s,
                             rhs=H2c[:, 0:N3], start=True, stop=True,
                             tile_position=(0, 96))
            stg = st_pool.tile([128, 512], f32, tag="st", name=f"stg{b}")
            nc.scalar.copy(out=stg[0:97, :], in_=lg[0:97, :])
            stg4 = stg[:].rearrange("(a b) f -> a b f", b=32)[:, 0:1, :]
            nc.gpsimd.dma_start(
                out=out[b:b + 1, :].rearrange("o (a f) -> o a f", a=4),
                in_=stg4)
            del state["H2"][b]

        # ---- 3-stage software pipeline over batches ----
        # W2(b-1) is emitted BEFORE S(b): the stream banks S(b) reuses
        # must already have their W2 writes registered with the tracker.
        for b in range(BPC + 2):
            if 1 <= b <= BPC:
                emit_W2(b - 1)
            if b < BPC:
                emit_S(b)
            if b >= 2:
                emit_W3(b - 2)
            if b < BPC:
                if b % 2 == 0 and (b // 2 + 4) < NCHUNK:
                    emit_gather(b // 2 + 4)
                # setup for the next chunk, one batch ahead of its use
                if b % 2 == 1 and (b // 2 + 1) < NCHUNK:
                    emit_chunk_setup(b // 2 + 1)

    nc.finalize()
    return nc


_CACHE: dict = {}


def _get_module() -> bass.Bass:
    if "nc" not in _CACHE:
        _CACHE["nc"] = _build_module()
    return _CACHE["nc"]


def _np_noop(dummy, W1, b1, W2, b2, W3, b3) -> float:
    dt = np.float64
    d1 = np.maximum(np.asarray(dummy, dt) @ np.asarray(W1, dt) + np.asarray(b1, dt), 0.0)
    d2 = np.maximum(d1 @ np.asarray(W2, dt) + np.asarray(b2, dt), 0.0)
    return float((d2 @ np.asarray(W3, dt) + np.asarray(b3, dt)).reshape(-1)[0])


def _make_in_maps(inputs):
    import ml_dtypes
    bf = ml_dtypes.bfloat16

    ops_emb = np.asarray(inputs["ops_emb"], dtype=np.float32).astype(bf)
    ma_emb = np.asarray(inputs["ma_emb"], dtype=np.float32).astype(bf)
    next_op = np.asarray(inputs["next_op"])
    W1 = np.asarray(inputs["W1"], dtype=np.float32).astype(bf)
    b1 = np.asarray(inputs["b1"], dtype=np.float32)
    W2 = np.asarray(inputs["W2"], dtype=np.float32).astype(bf)
    b2 = np.asarray(inputs["b2"], dtype=np.float32)
    W3 = np.asarray(inputs["W3"], dtype=np.float32).astype(bf)
    smat = _build_smat().astype(bf)

    wall = np.zeros((128, 385), dtype=bf)
    wall[:, 0:128] = W1[:E]
    wall[:, 128:256] = W1[E:]
    wall[:, 256:384] = W2
    wall[:, 384:385] = W3
    bvec = np.stack([b1, b2], axis=1)          # [128, 2] f32

    in_maps = []
    for core in range(NCORES):
        bsl = slice(core * BPC, (core + 1) * BPC)
        no = np.asarray(next_op[bsl], dtype=np.int64)          # [BPC, 50]
        gidx = np.zeros((BPC, PB), np.int64)
        gidx[:, :N_JOBS] = no + (np.arange(BPC, dtype=np.int64)[:, None] * N_OPS)
        idx2d = np.ascontiguousarray(
            gidx.reshape(NCHUNK, 128).T.astype(np.int32))      # [128, NCHUNK]
        in_maps.append({
            "ops": np.ascontiguousarray(ops_emb[bsl].reshape(BPC * N_OPS, E)),
            "ma": np.ascontiguousarray(ma_emb[bsl].reshape(BPC * N_MA, E)),
            "idx": idx2d,
            "smat": smat,
            "wall": wall,
            "bvec": np.ascontiguousarray(bvec),
        })
    return in_maps


def _run(inputs, trace=False, **kw):
    action_mask = np.asarray(inputs["action_mask"])
    b3 = np.asarray(inputs["b3"], dtype=np.float32)
    noop = _np_noop(inputs["dummy"], inputs["W1"], inputs["b1"],
                    inputs["W2"], inputs["b2"], inputs["W3"], inputs["b3"])
    nc = _get_module()
    in_maps = _make_in_maps(inputs)
    res = run_bass_kernel_spmd(nc, in_maps, core_ids=list(range(NCORES)),
                               trace=trace, **kw)
    logits = np.concatenate([r["out"][:, :NPAIR] for r in res.results], axis=0)
    logits = (logits + b3.reshape(-1)[0]).astype(np.float32)
    logits[:, 0] = noop
    return (logits, action_mask), res


def kernel(**inputs):
    out, _ = _run(inputs)
    return out
